# revision 2
# baseline (speedup 1.0000x reference)
"""Trainium2 Bass kernel for BinaryMLP.

reference:
    h = relu(x @ sign(W1).T + b1)   # [B, 128], x: [B, 196]
    h = relu(h @ sign(W2).T + b2)   # [B, 128]
    h = relu(h @ sign(W3).T + b3)   # [B, 128]
    y = h @ W4.T + b4               # [B, 10]

Strategy (pure data parallel over 8 cores, 65536 rows each):
  - Host: transpose + bf16-cast the x shard -> xT [196, B_core] so the
    contraction dim lands on SBUF partitions and every DMA is contiguous.
    sign(W) is exact in bf16. b4 is added on host (avoids an extra pass
    on the 10-partition head output).
  - Device: stream 512-column batch tiles. Per tile: 2 matmuls for L1
    (K = 196 = 128 + 68), 1 each for L2/L3 (K = 128), ReLU+bias
    evacuations alternate between ScalarE (activation) and VectorE
    (tensor_scalar) to balance the two engines. The head (M = 10) is
    packed 4-tiles-at-a-time into a single PSUM bank with 4x column
    tiling (tile_position=(0, 32u)) so 4 head matmuls run concurrently
    and one copy evacuates all 4.
  - Output is written as yT [10, B_core] fp32 and untransposed on host.
"""

import numpy as np
import ml_dtypes

import concourse.bass as bass
import concourse.mybir as mybir
import concourse.tile as tile
from concourse import bacc
from concourse.bass_utils import run_bass_kernel_spmd

BF16 = ml_dtypes.bfloat16

B_FULL, D_IN, H, D_OUT = 524288, 196, 128, 10
N_CORES = 8
TB = 512          # batch tile = matmul free dim (one PSUM bank of fp32)
GROUP = 4         # batch tiles per head-pack group
K1A = 128
K1B = D_IN - K1A  # 68


def build_nc(b_core: int, n_cores: int = N_CORES):
    """Build the per-core Bass program (SPMD: same program on all cores)."""
    dt = mybir.dt
    nc = bacc.Bacc(
        "TRN2", target_bir_lowering=False, debug=False, num_devices=n_cores
    )

    xT = nc.dram_tensor("xT", [D_IN, b_core], dt.bfloat16, kind="ExternalInput").ap()
    w1t = nc.dram_tensor("w1t", [D_IN, H], dt.bfloat16, kind="ExternalInput").ap()
    w2t = nc.dram_tensor("w2t", [H, H], dt.bfloat16, kind="ExternalInput").ap()
    w3t = nc.dram_tensor("w3t", [H, H], dt.bfloat16, kind="ExternalInput").ap()
    w4t = nc.dram_tensor("w4t", [H, 32], dt.bfloat16, kind="ExternalInput").ap()
    b1d = nc.dram_tensor("b1", [H, 1], dt.float32, kind="ExternalInput").ap()
    b2d = nc.dram_tensor("b2", [H, 1], dt.float32, kind="ExternalInput").ap()
    b3d = nc.dram_tensor("b3", [H, 1], dt.float32, kind="ExternalInput").ap()
    yT = nc.dram_tensor("yT", [D_OUT, b_core], dt.float32, kind="ExternalOutput").ap()

    n_tiles = b_core // TB
    assert b_core % TB == 0 and n_tiles % GROUP == 0
    n_groups = n_tiles // GROUP

    relu = mybir.ActivationFunctionType.Relu

    with tile.TileContext(nc) as tc:
        with (
            tc.tile_pool(name="wpool", bufs=1) as wpool,
            tc.tile_pool(name="xa", bufs=3) as xa_pool,
            tc.tile_pool(name="xb", bufs=3) as xb_pool,
            tc.tile_pool(name="h", bufs=3) as h_pool,
            tc.tile_pool(name="h3", bufs=GROUP + 2) as h3_pool,
            tc.tile_pool(name="yo", bufs=2) as y_pool,
            tc.tile_pool(name="ps1", bufs=2, space="PSUM") as ps1,
            tc.tile_pool(name="ps2", bufs=2, space="PSUM") as ps2,
            tc.tile_pool(name="ps3", bufs=2, space="PSUM") as ps3,
            tc.tile_pool(name="ps4", bufs=2, space="PSUM") as ps4,
        ):
            # --- load weights/biases once ---
            w1a_sb = wpool.tile([K1A, H], dt.bfloat16)
            nc.sync.dma_start(w1a_sb[:], w1t[0:K1A, :])
            w1b_sb = wpool.tile([K1B, H], dt.bfloat16)
            nc.sync.dma_start(w1b_sb[:], w1t[K1A:D_IN, :])
            w2_sb = wpool.tile([H, H], dt.bfloat16)
            nc.sync.dma_start(w2_sb[:], w2t[:, :])
            w3_sb = wpool.tile([H, H], dt.bfloat16)
            nc.sync.dma_start(w3_sb[:], w3t[:, :])
            w4_sb = wpool.tile([H, 32], dt.bfloat16)
            nc.sync.dma_start(w4_sb[:], w4t[:, :])
            b1_sb = wpool.tile([H, 1], dt.float32)
            nc.sync.dma_start(b1_sb[:], b1d[:, :])
            b2_sb = wpool.tile([H, 1], dt.float32)
            nc.sync.dma_start(b2_sb[:], b2d[:, :])
            b3_sb = wpool.tile([H, 1], dt.float32)
            nc.sync.dma_start(b3_sb[:], b3d[:, :])

            def relu_evac(use_act, h_out, psum_in, bias_sb):
                if use_act:
                    nc.scalar.activation(h_out[:], psum_in[:], relu, bias=bias_sb[:])
                else:
                    nc.vector.tensor_scalar(
                        h_out[:],
                        psum_in[:],
                        bias_sb[:],
                        0.0,
                        mybir.AluOpType.add,
                        mybir.AluOpType.max,
                    )

            for g in range(n_groups):
                c0 = g * GROUP * TB
                xa = xa_pool.tile([K1A, GROUP * TB], dt.bfloat16)
                nc.sync.dma_start(xa[:], xT[0:K1A, c0 : c0 + GROUP * TB])
                xb = xb_pool.tile([K1B, GROUP * TB], dt.bfloat16)
                nc.sync.dma_start(xb[:], xT[K1A:D_IN, c0 : c0 + GROUP * TB])

                h3s = []
                for u in range(GROUP):
                    cs = slice(u * TB, (u + 1) * TB)
                    p1 = ps1.tile([H, TB], dt.float32)
                    nc.tensor.matmul(p1[:], w1a_sb[:], xa[:, cs], start=True, stop=False)
                    nc.tensor.matmul(p1[:], w1b_sb[:], xb[:, cs], start=False, stop=True)
                    h1 = h_pool.tile([H, TB], dt.bfloat16, tag="h1")
                    relu_evac(u % 2 == 0, h1, p1, b1_sb)

                    p2 = ps2.tile([H, TB], dt.float32)
                    nc.tensor.matmul(p2[:], w2_sb[:], h1[:], start=True, stop=True)
                    h2 = h_pool.tile([H, TB], dt.bfloat16, tag="h2")
                    relu_evac(u % 2 == 1, h2, p2, b2_sb)

                    p3 = ps3.tile([H, TB], dt.float32)
                    nc.tensor.matmul(p3[:], w3_sb[:], h2[:], start=True, stop=True)
                    h3 = h3_pool.tile([H, TB], dt.bfloat16, tag="h3")
                    relu_evac(u % 2 == 0, h3, p3, b3_sb)
                    h3s.append(h3)

                # head: 4x column tiling -> 4 concurrent M=10 matmuls in one bank
                p4 = ps4.tile([H, TB], dt.float32)
                for u in range(GROUP):
                    nc.tensor.matmul(
                        p4[32 * u : 32 * u + 32, :],
                        w4_sb[:],
                        h3s[u][:],
                        start=True,
                        stop=True,
                        tile_position=(0, 32 * u),
                    )
                ysb = y_pool.tile([H, TB], dt.float32)
                nc.scalar.copy(ysb[:], p4[:])
                for u in range(GROUP):
                    nc.sync.dma_start(
                        yT[:, c0 + u * TB : c0 + (u + 1) * TB],
                        ysb[32 * u : 32 * u + D_OUT, :],
                    )

    nc.compile()
    return nc


def _prep_core_inputs(x_shard: np.ndarray, weights: dict) -> dict:
    xT = np.ascontiguousarray(x_shard.T).astype(BF16)
    return {"xT": xT, **weights}


def _prep_weights(W1, b1, W2, b2, W3, b3, W4) -> dict:
    return {
        "w1t": np.ascontiguousarray(np.sign(W1).T).astype(BF16),
        "w2t": np.ascontiguousarray(np.sign(W2).T).astype(BF16),
        "w3t": np.ascontiguousarray(np.sign(W3).T).astype(BF16),
        "w4t": np.ascontiguousarray(
            np.concatenate([W4, np.zeros((32 - D_OUT, H), np.float32)], axis=0).T
        ).astype(BF16),
        "b1": b1.reshape(H, 1).astype(np.float32),
        "b2": b2.reshape(H, 1).astype(np.float32),
        "b3": b3.reshape(H, 1).astype(np.float32),
    }


_NC_CACHE: dict = {}


def run(x, W1, b1, W2, b2, W3, b3, W4, b4, trace=False, trace_kwargs=None):
    """Run the SPMD kernel on 8 cores; returns (y, BassKernelResults)."""
    x = np.asarray(x, dtype=np.float32)
    b_total = x.shape[0]
    assert b_total % N_CORES == 0
    b_core = b_total // N_CORES

    key = b_core
    if key not in _NC_CACHE:
        _NC_CACHE[key] = build_nc(b_core)
    nc = _NC_CACHE[key]

    weights = _prep_weights(
        np.asarray(W1), np.asarray(b1), np.asarray(W2), np.asarray(b2),
        np.asarray(W3), np.asarray(b3), np.asarray(W4),
    )
    in_maps = [
        _prep_core_inputs(x[c * b_core : (c + 1) * b_core], weights)
        for c in range(N_CORES)
    ]
    res = run_bass_kernel_spmd(
        nc,
        in_maps,
        list(range(N_CORES)),
        trace=trace,
        **(trace_kwargs or {}),
    )
    b4f = np.asarray(b4, dtype=np.float32)
    y = np.empty((b_total, D_OUT), dtype=np.float32)
    for c in range(N_CORES):
        y[c * b_core : (c + 1) * b_core] = res.results[c]["yT"].T
    y += b4f
    return y, res


def kernel(x, W1, b1, W2, b2, W3, b3, W4, b4):
    y, _ = run(x, W1, b1, W2, b2, W3, b3, W4, b4)
    return y


# revision 6
# speedup vs baseline: 1.0501x; 1.0501x over previous
"""Trainium2 Bass kernel for BinaryMLP.

reference:
    h = relu(x @ sign(W1).T + b1)   # [B, 128], x: [B, 196]
    h = relu(h @ sign(W2).T + b2)   # [B, 128]
    h = relu(h @ sign(W3).T + b3)   # [B, 128]
    y = h @ W4.T + b4               # [B, 10]

Strategy (pure data parallel over 8 cores, 65536 rows each):
  - Host: transpose + bf16-cast the x shard -> xT [196, B_core] so the
    contraction dim lands on SBUF partitions and every DMA is contiguous.
    sign(W) is exact in bf16. b4 is added on host.
  - Device: 512-column batch tiles, processed in weight-paired twos so
    consecutive matmuls share a stationary operand (hides LDWEIGHTS), with
    2-bank PSUM tensors so each ReLU+bias evacuation covers 1024 columns
    (amortizes the fixed per-op engine cost). Evacuations alternate between
    ScalarE (activation Relu w/ per-partition bias) and VectorE
    (tensor_scalar add+max) to balance the engines.
  - Head (M=10): packed 8 tiles per PSUM bank using 4x column tiling
    (tile_position=(0,32u)) x 2 accumulated zero-masked W4 variants
    (rows 0-9 / 10-19 of each 32-partition strip), so eight N=512 head
    matmuls cost ~2 matmul slots on the PE and a single [128,512] copy
    evacuates all eight.
  - Output is written as yT [10, B_core] fp32 and untransposed on host.
"""

import numpy as np
import ml_dtypes

import concourse.bass as bass
import concourse.mybir as mybir
import concourse.tile as tile
from concourse import bacc
from concourse.bass_utils import run_bass_kernel_spmd

BF16 = ml_dtypes.bfloat16

B_FULL, D_IN, H, D_OUT = 524288, 196, 128, 10
N_CORES = 8
TB = 512          # batch tile = matmul free dim (one PSUM bank of fp32)
PACK = 8          # tiles per head pack / DMA load / store group
K1A = 128
K1B = D_IN - K1A  # 68


def build_nc(b_core: int, n_cores: int = N_CORES):
    """Build the per-core Bass program (SPMD: same program on all cores)."""
    dt = mybir.dt
    nc = bacc.Bacc(
        "TRN2", target_bir_lowering=False, debug=False, num_devices=n_cores
    )

    xT = nc.dram_tensor("xT", [D_IN, b_core], dt.bfloat16, kind="ExternalInput").ap()
    w1t = nc.dram_tensor("w1t", [D_IN, H], dt.bfloat16, kind="ExternalInput").ap()
    w2t = nc.dram_tensor("w2t", [H, H], dt.bfloat16, kind="ExternalInput").ap()
    w3t = nc.dram_tensor("w3t", [H, H], dt.bfloat16, kind="ExternalInput").ap()
    w4a = nc.dram_tensor("w4a", [H, 32], dt.bfloat16, kind="ExternalInput").ap()
    w4b = nc.dram_tensor("w4b", [H, 32], dt.bfloat16, kind="ExternalInput").ap()
    b1d = nc.dram_tensor("b1", [H, 1], dt.float32, kind="ExternalInput").ap()
    b2d = nc.dram_tensor("b2", [H, 1], dt.float32, kind="ExternalInput").ap()
    b3d = nc.dram_tensor("b3", [H, 1], dt.float32, kind="ExternalInput").ap()
    yT = nc.dram_tensor("yT", [D_OUT, b_core], dt.float32, kind="ExternalOutput").ap()

    n_tiles = b_core // TB
    assert b_core % (PACK * TB) == 0
    n_packs = n_tiles // PACK

    relu = mybir.ActivationFunctionType.Relu

    with tile.TileContext(nc) as tc:
        with (
            tc.tile_pool(name="wpool", bufs=1) as wpool,
            tc.tile_pool(name="xa", bufs=2) as xa_pool,
            tc.tile_pool(name="xb", bufs=2) as xb_pool,
            tc.tile_pool(name="h", bufs=3) as h_pool,
            tc.tile_pool(name="h3", bufs=6) as h3_pool,
            tc.tile_pool(name="yo", bufs=2) as y_pool,
            tc.tile_pool(name="ps1", bufs=1, space="PSUM") as ps1,
            tc.tile_pool(name="ps2", bufs=1, space="PSUM") as ps2,
            tc.tile_pool(name="ps3", bufs=1, space="PSUM") as ps3,
            tc.tile_pool(name="ps4", bufs=2, space="PSUM") as ps4,
        ):
            # --- load weights/biases once ---
            w1a_sb = wpool.tile([K1A, H], dt.bfloat16)
            nc.sync.dma_start(w1a_sb[:], w1t[0:K1A, :])
            w1b_sb = wpool.tile([K1B, H], dt.bfloat16)
            nc.sync.dma_start(w1b_sb[:], w1t[K1A:D_IN, :])
            w2_sb = wpool.tile([H, H], dt.bfloat16)
            nc.sync.dma_start(w2_sb[:], w2t[:, :])
            w3_sb = wpool.tile([H, H], dt.bfloat16)
            nc.sync.dma_start(w3_sb[:], w3t[:, :])
            w4_sb = [
                wpool.tile([H, 32], dt.bfloat16, tag=f"w4_{j}", name=f"w4_{j}")
                for j in range(2)
            ]
            nc.sync.dma_start(w4_sb[0][:], w4a[:, :])
            nc.sync.dma_start(w4_sb[1][:], w4b[:, :])
            b_sb = []
            for j, bd in enumerate((b1d, b2d, b3d)):
                b = wpool.tile([H, 1], dt.float32, tag=f"b_{j}", name=f"b_{j}")
                nc.sync.dma_start(b[:], bd[:, :])
                b_sb.append(b)

            def relu_evac(use_act, h_out, psum_in, bias_sb):
                if use_act:
                    nc.scalar.activation(h_out[:], psum_in[:], relu, bias=bias_sb[:])
                else:
                    nc.vector.tensor_scalar(
                        h_out[:],
                        psum_in[:],
                        bias_sb[:],
                        0.0,
                        mybir.AluOpType.add,
                        mybir.AluOpType.max,
                    )

            W = PACK * TB  # columns per load/store group
            for pk in range(n_packs):
                c0 = pk * W
                xa = xa_pool.tile([K1A, W], dt.bfloat16)
                nc.sync.dma_start(xa[:], xT[0:K1A, c0 : c0 + W])
                xb = xb_pool.tile([K1B, W], dt.bfloat16)
                nc.scalar.dma_start(xb[:], xT[K1A:D_IN, c0 : c0 + W])

                h3s = []
                for pr in range(PACK // 2):
                    o = pr * 2 * TB
                    s0 = slice(o, o + TB)
                    s1 = slice(o + TB, o + 2 * TB)
                    pi = pk * (PACK // 2) + pr  # global pair index
                    ea = pi % 2 == 0  # engine alternation

                    p1 = ps1.tile([H, 2 * TB], dt.float32)
                    nc.tensor.matmul(p1[:, 0:TB], w1a_sb[:], xa[:, s0], start=True, stop=False)
                    nc.tensor.matmul(p1[:, TB:], w1a_sb[:], xa[:, s1], start=True, stop=False)
                    nc.tensor.matmul(p1[:, 0:TB], w1b_sb[:], xb[:, s0], start=False, stop=True)
                    nc.tensor.matmul(p1[:, TB:], w1b_sb[:], xb[:, s1], start=False, stop=True)
                    h1 = h_pool.tile([H, 2 * TB], dt.bfloat16, tag="h1")
                    relu_evac(ea, h1, p1, b_sb[0])

                    p2 = ps2.tile([H, 2 * TB], dt.float32)
                    nc.tensor.matmul(p2[:, 0:TB], w2_sb[:], h1[:, 0:TB], start=True, stop=True)
                    nc.tensor.matmul(p2[:, TB:], w2_sb[:], h1[:, TB:], start=True, stop=True)
                    h2 = h_pool.tile([H, 2 * TB], dt.bfloat16, tag="h2")
                    relu_evac(not ea, h2, p2, b_sb[1])

                    p3 = ps3.tile([H, 2 * TB], dt.float32)
                    nc.tensor.matmul(p3[:, 0:TB], w3_sb[:], h2[:, 0:TB], start=True, stop=True)
                    nc.tensor.matmul(p3[:, TB:], w3_sb[:], h2[:, TB:], start=True, stop=True)
                    h3 = h3_pool.tile([H, 2 * TB], dt.bfloat16, tag="h3")
                    relu_evac(ea, h3, p3, b_sb[2])
                    h3s.append(h3)

                # head: tile t of pack -> strip u = t//2, variant j = t%2.
                # Variant j writes rows 32u+10j..+10; M=32 keeps the whole
                # strip initialized (zero weight columns elsewhere).
                p4 = ps4.tile([H, TB], dt.float32)
                for j in range(2):
                    for u in range(4):
                        t = 2 * u + j
                        h3t = h3s[t // 2][:, (t % 2) * TB : (t % 2 + 1) * TB]
                        nc.tensor.matmul(
                            p4[32 * u : 32 * u + 32, :],
                            w4_sb[j][:],
                            h3t,
                            start=(j == 0),
                            stop=(j == 1),
                            tile_position=(0, 32 * u),
                            skip_group_check=True,
                        )
                ysb = y_pool.tile([H, TB], dt.float32)
                nc.scalar.copy(ysb[:], p4[:])
                # store: rows 32u+10j..+10 hold tile (2u+j)'s y.T
                for u in range(4):
                    for j in range(2):
                        t = 2 * u + j
                        nc.sync.dma_start(
                            yT[:, c0 + t * TB : c0 + (t + 1) * TB],
                            ysb[32 * u + 10 * j : 32 * u + 10 * j + D_OUT, :],
                        )

    nc.compile()
    return nc


def _prep_core_inputs(x_shard: np.ndarray, weights: dict) -> dict:
    xT = np.ascontiguousarray(x_shard.T).astype(BF16)
    return {"xT": xT, **weights}


def _prep_weights(W1, b1, W2, b2, W3, b3, W4) -> dict:
    w4a = np.zeros((32, H), np.float32)
    w4a[0:D_OUT] = W4
    w4b = np.zeros((32, H), np.float32)
    w4b[D_OUT : 2 * D_OUT] = W4
    return {
        "w1t": np.ascontiguousarray(np.sign(W1).T).astype(BF16),
        "w2t": np.ascontiguousarray(np.sign(W2).T).astype(BF16),
        "w3t": np.ascontiguousarray(np.sign(W3).T).astype(BF16),
        "w4a": np.ascontiguousarray(w4a.T).astype(BF16),
        "w4b": np.ascontiguousarray(w4b.T).astype(BF16),
        "b1": b1.reshape(H, 1).astype(np.float32),
        "b2": b2.reshape(H, 1).astype(np.float32),
        "b3": b3.reshape(H, 1).astype(np.float32),
    }


_NC_CACHE: dict = {}


def run(x, W1, b1, W2, b2, W3, b3, W4, b4, trace=False, trace_kwargs=None):
    """Run the SPMD kernel on 8 cores; returns (y, BassKernelResults)."""
    x = np.asarray(x, dtype=np.float32)
    b_total = x.shape[0]
    assert b_total % N_CORES == 0
    b_core = b_total // N_CORES

    key = b_core
    if key not in _NC_CACHE:
        _NC_CACHE[key] = build_nc(b_core)
    nc = _NC_CACHE[key]

    weights = _prep_weights(
        np.asarray(W1), np.asarray(b1), np.asarray(W2), np.asarray(b2),
        np.asarray(W3), np.asarray(b3), np.asarray(W4),
    )
    in_maps = [
        _prep_core_inputs(x[c * b_core : (c + 1) * b_core], weights)
        for c in range(N_CORES)
    ]
    res = run_bass_kernel_spmd(
        nc,
        in_maps,
        list(range(N_CORES)),
        trace=trace,
        **(trace_kwargs or {}),
    )
    b4f = np.asarray(b4, dtype=np.float32)
    y = np.empty((b_total, D_OUT), dtype=np.float32)
    for c in range(N_CORES):
        y[c * b_core : (c + 1) * b_core] = res.results[c]["yT"].T
    y += b4f
    return y, res


def kernel(x, W1, b1, W2, b2, W3, b3, W4, b4):
    y, _ = run(x, W1, b1, W2, b2, W3, b3, W4, b4)
    return y


# revision 8
# speedup vs baseline: 1.2932x; 1.2315x over previous
"""Trainium2 Bass kernel for BinaryMLP.

reference:
    h = relu(x @ sign(W1).T + b1)   # [B, 128], x: [B, 196]
    h = relu(h @ sign(W2).T + b2)   # [B, 128]
    h = relu(h @ sign(W3).T + b3)   # [B, 128]
    y = h @ W4.T + b4               # [B, 10]

Strategy (pure data parallel over 8 cores, 65536 rows each):
  - Host: transpose + bf16-cast the x shard -> xT [196, B_core] so the
    contraction dim lands on SBUF partitions and every DMA is contiguous.
    sign(W) is exact in bf16. b4 is added on host.
  - Device: 512-column batch tiles, processed in weight-paired twos
    (consecutive matmuls share the stationary operand, so LDWEIGHTS is
    hidden), with 2-bank PSUM tensors so each ReLU+bias evacuation covers
    1024 columns. Evacuations alternate between ScalarE and VectorE.
  - Software pipelining: layer stages are emitted with a 2-pair skew
    (L2 of pair i-2, L1 of pair i, L3 of pair i-4 per step) so the
    in-order PE always has independent matmuls to run while evacuations
    complete -> no PE stalls, HAM stays warm.
  - Head (M=10): packed 8 tiles per PSUM bank using 4x column tiling
    (tile_position=(0,32u)) x 2 accumulated zero-masked W4 variants, so
    eight N=512 head matmuls cost ~2 matmul slots and one [128,512] copy
    evacuates all eight. Output stays in the strip layout yTS[128, .]
    (rows 32u+10j+p) -> one [128,512] store per 8 tiles; the host
    unscrambles and adds b4.
"""

import numpy as np
import ml_dtypes

import concourse.bass as bass
import concourse.mybir as mybir
import concourse.tile as tile
from concourse import bacc
from concourse.bass_utils import run_bass_kernel_spmd

BF16 = ml_dtypes.bfloat16

B_FULL, D_IN, H, D_OUT = 524288, 196, 128, 10
N_CORES = 8
TB = 512          # batch tile = matmul free dim (one PSUM bank of fp32)
PACK = 8          # tiles per head pack / DMA load / store group
K1A = 128
K1B = D_IN - K1A  # 68


def build_nc(b_core: int, n_cores: int = N_CORES):
    """Build the per-core Bass program (SPMD: same program on all cores)."""
    dt = mybir.dt
    nc = bacc.Bacc(
        "TRN2", target_bir_lowering=False, debug=False, num_devices=n_cores
    )

    n_tiles = b_core // TB
    assert b_core % (PACK * TB) == 0
    n_packs = n_tiles // PACK
    n_pairs = n_tiles // 2

    xT = nc.dram_tensor("xT", [D_IN, b_core], dt.bfloat16, kind="ExternalInput").ap()
    w1t = nc.dram_tensor("w1t", [D_IN, H], dt.bfloat16, kind="ExternalInput").ap()
    w2t = nc.dram_tensor("w2t", [H, H], dt.bfloat16, kind="ExternalInput").ap()
    w3t = nc.dram_tensor("w3t", [H, H], dt.bfloat16, kind="ExternalInput").ap()
    w4a = nc.dram_tensor("w4a", [H, 32], dt.bfloat16, kind="ExternalInput").ap()
    w4b = nc.dram_tensor("w4b", [H, 32], dt.bfloat16, kind="ExternalInput").ap()
    b1d = nc.dram_tensor("b1", [H, 1], dt.float32, kind="ExternalInput").ap()
    b2d = nc.dram_tensor("b2", [H, 1], dt.float32, kind="ExternalInput").ap()
    b3d = nc.dram_tensor("b3", [H, 1], dt.float32, kind="ExternalInput").ap()
    # strip-layout output: row 32u+10j+p, cols pk*TB+c  <->  y[(pk*8+2u+j)*TB+c, p]
    yTS = nc.dram_tensor(
        "yTS", [H, n_packs * TB], dt.float32, kind="ExternalOutput"
    ).ap()

    relu = mybir.ActivationFunctionType.Relu

    with tile.TileContext(nc) as tc:
        with (
            tc.tile_pool(name="wpool", bufs=1) as wpool,
            tc.tile_pool(name="xa", bufs=3) as xa_pool,
            tc.tile_pool(name="xb", bufs=3) as xb_pool,
            tc.tile_pool(name="h1p", bufs=4) as h1_pool,
            tc.tile_pool(name="h2p", bufs=4) as h2_pool,
            tc.tile_pool(name="h3p", bufs=10) as h3_pool,
            tc.tile_pool(name="yo", bufs=2) as y_pool,
            tc.tile_pool(name="ps1", bufs=1, space="PSUM") as ps1,
            tc.tile_pool(name="ps2", bufs=1, space="PSUM") as ps2,
            tc.tile_pool(name="ps3", bufs=1, space="PSUM") as ps3,
            tc.tile_pool(name="ps4", bufs=2, space="PSUM") as ps4,
        ):
            # --- load weights/biases once ---
            w1a_sb = wpool.tile([K1A, H], dt.bfloat16)
            nc.sync.dma_start(w1a_sb[:], w1t[0:K1A, :])
            w1b_sb = wpool.tile([K1B, H], dt.bfloat16)
            nc.sync.dma_start(w1b_sb[:], w1t[K1A:D_IN, :])
            w2_sb = wpool.tile([H, H], dt.bfloat16)
            nc.sync.dma_start(w2_sb[:], w2t[:, :])
            w3_sb = wpool.tile([H, H], dt.bfloat16)
            nc.sync.dma_start(w3_sb[:], w3t[:, :])
            w4_sb = [
                wpool.tile([H, 32], dt.bfloat16, tag=f"w4_{j}", name=f"w4_{j}")
                for j in range(2)
            ]
            nc.sync.dma_start(w4_sb[0][:], w4a[:, :])
            nc.sync.dma_start(w4_sb[1][:], w4b[:, :])
            b_sb = []
            for j, bd in enumerate((b1d, b2d, b3d)):
                b = wpool.tile([H, 1], dt.float32, tag=f"b_{j}", name=f"b_{j}")
                nc.sync.dma_start(b[:], bd[:, :])
                b_sb.append(b)

            def relu_evac(use_act, h_out, psum_in, bias_sb):
                if use_act:
                    nc.scalar.activation(h_out[:], psum_in[:], relu, bias=bias_sb[:])
                else:
                    nc.vector.tensor_scalar(
                        h_out[:],
                        psum_in[:],
                        bias_sb[:],
                        0.0,
                        mybir.AluOpType.add,
                        mybir.AluOpType.max,
                    )

            W = PACK * TB  # columns per load group
            xa_t: dict = {}
            xb_t: dict = {}
            h1_t: dict = {}
            h2_t: dict = {}
            h3_t: dict = {}

            def emit_load(pk):
                c0 = pk * W
                xa = xa_pool.tile([K1A, W], dt.bfloat16, tag="xa", name=f"xa_{pk}")
                nc.sync.dma_start(xa[:], xT[0:K1A, c0 : c0 + W])
                xb = xb_pool.tile([K1B, W], dt.bfloat16, tag="xb", name=f"xb_{pk}")
                nc.sync.dma_start(xb[:], xT[K1A:D_IN, c0 : c0 + W])
                xa_t[pk], xb_t[pk] = xa, xb

            def stage_A(i):  # L1 matmuls + evac for pair i
                pk, pr = divmod(i, PACK // 2)
                o = pr * 2 * TB
                s0, s1 = slice(o, o + TB), slice(o + TB, o + 2 * TB)
                xa, xb = xa_t[pk], xb_t[pk]
                p1 = ps1.tile([H, 2 * TB], dt.float32, tag="p1", name=f"p1_{i}")
                nc.tensor.matmul(p1[:, 0:TB], w1a_sb[:], xa[:, s0], start=True, stop=False)
                nc.tensor.matmul(p1[:, TB:], w1a_sb[:], xa[:, s1], start=True, stop=False)
                nc.tensor.matmul(p1[:, 0:TB], w1b_sb[:], xb[:, s0], start=False, stop=True)
                nc.tensor.matmul(p1[:, TB:], w1b_sb[:], xb[:, s1], start=False, stop=True)
                h1 = h1_pool.tile([H, 2 * TB], dt.bfloat16, tag="h1", name=f"h1_{i}")
                relu_evac(i % 2 == 0, h1, p1, b_sb[0])
                h1_t[i] = h1

            def stage_B(i):  # L2
                h1 = h1_t.pop(i)
                p2 = ps2.tile([H, 2 * TB], dt.float32, tag="p2", name=f"p2_{i}")
                nc.tensor.matmul(p2[:, 0:TB], w2_sb[:], h1[:, 0:TB], start=True, stop=True)
                nc.tensor.matmul(p2[:, TB:], w2_sb[:], h1[:, TB:], start=True, stop=True)
                h2 = h2_pool.tile([H, 2 * TB], dt.bfloat16, tag="h2", name=f"h2_{i}")
                relu_evac(i % 2 == 1, h2, p2, b_sb[1])
                h2_t[i] = h2

            def stage_C(i):  # L3
                h2 = h2_t.pop(i)
                p3 = ps3.tile([H, 2 * TB], dt.float32, tag="p3", name=f"p3_{i}")
                nc.tensor.matmul(p3[:, 0:TB], w3_sb[:], h2[:, 0:TB], start=True, stop=True)
                nc.tensor.matmul(p3[:, TB:], w3_sb[:], h2[:, TB:], start=True, stop=True)
                h3 = h3_pool.tile([H, 2 * TB], dt.bfloat16, tag="h3", name=f"h3_{i}")
                relu_evac(i % 2 == 0, h3, p3, b_sb[2])
                h3_t[i] = h3

            def stage_H(pk):  # head pack over pairs 4pk..4pk+3, one store
                h3s = [h3_t.pop(4 * pk + q) for q in range(PACK // 2)]
                p4 = ps4.tile([H, TB], dt.float32, tag="p4", name=f"p4_{pk}")
                for j in range(2):
                    for u in range(4):
                        t = 2 * u + j
                        h3t = h3s[t // 2][:, (t % 2) * TB : (t % 2 + 1) * TB]
                        nc.tensor.matmul(
                            p4[32 * u : 32 * u + 32, :],
                            w4_sb[j][:],
                            h3t,
                            start=(j == 0),
                            stop=(j == 1),
                            tile_position=(0, 32 * u),
                            skip_group_check=True,
                        )
                ysb = y_pool.tile([H, TB], dt.float32, tag="ysb", name=f"ysb_{pk}")
                nc.scalar.copy(ysb[:], p4[:])
                nc.sync.dma_start(yTS[:, pk * TB : (pk + 1) * TB], ysb[:])

            # --- software-pipelined emission ---
            emit_load(0)
            if n_packs > 1:
                emit_load(1)
            for step in range(n_pairs + 4):
                # prefetch pack (step+6)//4 at steps 2, 6, 10, ...
                if step % 4 == 2 and (step + 6) // 4 < n_packs:
                    emit_load((step + 6) // 4)
                ib = step - 2
                if 0 <= ib < n_pairs:
                    stage_B(ib)
                if step < n_pairs:
                    stage_A(step)
                ic = step - 4
                if 0 <= ic < n_pairs:
                    stage_C(ic)
                    if ic % 4 == 3:
                        stage_H(ic // 4)

    nc.compile()
    return nc


def _prep_core_inputs(x_shard: np.ndarray, weights: dict) -> dict:
    xT = np.ascontiguousarray(x_shard.T).astype(BF16)
    return {"xT": xT, **weights}


def _prep_weights(W1, b1, W2, b2, W3, b3, W4) -> dict:
    w4a = np.zeros((32, H), np.float32)
    w4a[0:D_OUT] = W4
    w4b = np.zeros((32, H), np.float32)
    w4b[D_OUT : 2 * D_OUT] = W4
    return {
        "w1t": np.ascontiguousarray(np.sign(W1).T).astype(BF16),
        "w2t": np.ascontiguousarray(np.sign(W2).T).astype(BF16),
        "w3t": np.ascontiguousarray(np.sign(W3).T).astype(BF16),
        "w4a": np.ascontiguousarray(w4a.T).astype(BF16),
        "w4b": np.ascontiguousarray(w4b.T).astype(BF16),
        "b1": b1.reshape(H, 1).astype(np.float32),
        "b2": b2.reshape(H, 1).astype(np.float32),
        "b3": b3.reshape(H, 1).astype(np.float32),
    }


def _unscramble(yTS: np.ndarray, b_core: int) -> np.ndarray:
    """yTS [128, n_packs*TB] strip layout -> y_core [b_core, 10]."""
    n_packs = b_core // (PACK * TB)
    # yTS[32u+10j+p, pk*TB+c] = y[(pk*8+2u+j)*TB + c, p]
    v = yTS.reshape(4, 32, n_packs, TB)[:, :20]  # [u, 10j+p, pk, c]
    v = v.reshape(4, 2, 10, n_packs, TB)  # [u, j, p, pk, c]
    # -> y[pk, u, j, c, p]
    y = v.transpose(3, 0, 1, 4, 2).reshape(b_core, D_OUT)
    return y


_NC_CACHE: dict = {}


def run(x, W1, b1, W2, b2, W3, b3, W4, b4, trace=False, trace_kwargs=None):
    """Run the SPMD kernel on 8 cores; returns (y, BassKernelResults)."""
    x = np.asarray(x, dtype=np.float32)
    b_total = x.shape[0]
    assert b_total % N_CORES == 0
    b_core = b_total // N_CORES

    key = b_core
    if key not in _NC_CACHE:
        _NC_CACHE[key] = build_nc(b_core)
    nc = _NC_CACHE[key]

    weights = _prep_weights(
        np.asarray(W1), np.asarray(b1), np.asarray(W2), np.asarray(b2),
        np.asarray(W3), np.asarray(b3), np.asarray(W4),
    )
    in_maps = [
        _prep_core_inputs(x[c * b_core : (c + 1) * b_core], weights)
        for c in range(N_CORES)
    ]
    res = run_bass_kernel_spmd(
        nc,
        in_maps,
        list(range(N_CORES)),
        trace=trace,
        **(trace_kwargs or {}),
    )
    b4f = np.asarray(b4, dtype=np.float32)
    y = np.empty((b_total, D_OUT), dtype=np.float32)
    for c in range(N_CORES):
        y[c * b_core : (c + 1) * b_core] = _unscramble(res.results[c]["yTS"], b_core)
    y += b4f
    return y, res


def kernel(x, W1, b1, W2, b2, W3, b3, W4, b4):
    y, _ = run(x, W1, b1, W2, b2, W3, b3, W4, b4)
    return y
